# revision 34
# baseline (speedup 1.0000x reference)
"""Cross-attention kernel for Trainium2, 8 NeuronCores.

Sharding (data + head parallel, per the problem's sharding hint):
  core c in 0..7 -> batch b = c // 4, head-pair hp = c % 4.
  Each core computes attention for its batch with 2 of the 8 heads
  (a 128-wide slice of the 512 hidden features), then the partial
  out-projection  attn_out_slice @ Wo[slice, :].  The host sums the 4
  partials per batch and adds bo (the "all-reduce" / unshard step).

Device-side dataflow per core (all matmul operands bf16):
  qT[128, N] = Wq_sl.T @ x.T          (contraction over D=1024 in 8 chunks)
  kT[128, M] = Wk_sl.T @ ctx.T
  vT[128, M] = Wv_sl.T @ ctx.T
  V_aug[m,65] = PE-transpose of vT per head + ones column
  per n-chunk s (512 cols), per m-chunk mc (128 rows):
     St[m 128, n 1024] = [kT_h0_mc.T @ qT_h0_s | kT_h1_mc.T @ qT_h1_s]
         (two concurrent matmuls on PE row-groups 0-63 / 64-127)
     Pt = exp(St * 1/8)               (ScalarE, one op per m-chunk)
     Oaug_h[65, 512] += V_aug_h_mc.T @ Pt_h                (PSUM accum)
  row 64 of Oaug = softmax denominators; OT[h*64:, s] = Oaug[0:64]/denom
  out_p[n 128, 1024] = OT_ntile.T @ Wo_sl  (bf16 partial, host adds bo)

Schedule notes (from trace analysis of the previous version):
  - Inputs are host-swizzled so each seq-chunk is ONE contiguous-line
    dma_start (128 x 8KB descriptors fan out over all 16 DMA engines);
    dma_start dispatch costs ~700ns serialized on its issuing engine,
    so few/large DMAs beat many/small ones.
  - The exp ACTIVATEs are the serial bottleneck (~86us on ScalarE);
    everything else (projections, PV, out-proj, DMAs) is interleaved
    into the attention slots so the ACT stream stays dense.
  - Dummy matmuls at t=0 warm the PE HAM clock gate (1.2 -> 2.4 GHz);
    a dummy exp preloads the ScalarE table set during the DMA head.
"""

import numpy as np

import concourse.bass as bass
import concourse.tile as tile
from concourse import bacc, mybir
from concourse.masks import make_identity

F32 = mybir.dt.float32
BF16 = mybir.dt.bfloat16

D = 1024      # model dim (contraction for projections)
SEQ = 2048    # n == m
F = 128       # features per core (2 heads x 64)
DH = 64       # head dim
NS = SEQ // 512   # 4 n-chunks of 512
NK = D // 128     # 8 contraction chunks
NM = SEQ // 128   # 16 m-chunks of 128
VPAD = 72         # PV weight row padded to 16B-aligned stride (bf16)
SCALE = DH ** -0.5
NWARM = 12        # PE warm-up dummy matmuls (~5us at 1.2GHz)


def build_nc():
    nc = bacc.Bacc("TRN2", target_bir_lowering=False, debug=False)

    # x / context arrive host-swizzled: row s*128+p, col k*512+j holds
    # xT[k*128+p, s*512+j], so the tile for seq-chunk s is one DMA with
    # 8KB contiguous lines.
    xs_d = nc.dram_tensor("xs", [NS * 128, NK * 512], BF16, kind="ExternalInput")
    cs_d = nc.dram_tensor("cs", [NS * 128, NK * 512], BF16, kind="ExternalInput")
    # wq/wk/wv pre-swizzled: [128, NK*128], column block k = W[k*128:(k+1)*128, :].T
    wq_d = nc.dram_tensor("wq", [128, NK * 128], BF16, kind="ExternalInput")
    wk_d = nc.dram_tensor("wk", [128, NK * 128], BF16, kind="ExternalInput")
    wv_d = nc.dram_tensor("wv", [128, NK * 128], BF16, kind="ExternalInput")
    wo_d = nc.dram_tensor("wo", [F, D], BF16, kind="ExternalInput")
    out_d = nc.dram_tensor("out_p", [SEQ, D], BF16, kind="ExternalOutput")

    with tile.TileContext(nc) as tc:
        _emit(tc, nc, xs_d, cs_d, wq_d, wk_d, wv_d, wo_d, out_d)
    nc.compile()
    return nc


def _emit(tc, nc, xs_d, cs_d, wq_d, wk_d, wv_d, wo_d, out_d):
    from contextlib import ExitStack

    ctx = ExitStack()
    wpool = ctx.enter_context(tc.tile_pool(name="wpool", bufs=1))
    big = ctx.enter_context(tc.tile_pool(name="big", bufs=1))
    ptp = ctx.enter_context(tc.tile_pool(name="ptp", bufs=4))
    ostage = ctx.enter_context(tc.tile_pool(name="ostage", bufs=2))
    opool = ctx.enter_context(tc.tile_pool(name="opool", bufs=4))
    dscr = ctx.enter_context(tc.tile_pool(name="dscr", bufs=2, space="DRAM"))
    ps_small = ctx.enter_context(tc.tile_pool(name="ps_small", bufs=2, space="PSUM"))
    ps_st = ctx.enter_context(tc.tile_pool(name="ps_st", bufs=2, space="PSUM"))
    ps_oaug = ctx.enter_context(tc.tile_pool(name="ps_oaug", bufs=2, space="PSUM"))

    # ---- constants ----
    ident = wpool.tile([128, 128], F32, name="ident")
    make_identity(nc, ident)
    zbias = wpool.tile([128, 1], F32, name="zbias")
    nc.vector.memset(zbias, 0.0)
    junkw = wpool.tile([128, 128], BF16, name="junkw")
    nc.vector.memset(junkw, 0.0)
    junkm = wpool.tile([128, 512], BF16, name="junkm")
    nc.vector.memset(junkm, 0.0)

    # preload the exp table set on ScalarE while DMAs stream
    act_warm = wpool.tile([128, 1], F32, name="act_warm")
    nc.scalar.activation(
        out=act_warm, in_=zbias,
        func=mybir.ActivationFunctionType.Exp, bias=zbias, scale=1.0,
    )

    # ---- input DMA dispatches (order == arrival priority) ----
    xs_t = [big.tile([128, NK, 512], BF16, name=f"xs{s}", tag=f"xs{s}")
            for s in range(NS)]
    # cs is half-chunk-major: [128, half, k, 256]
    cs_t = [big.tile([128, 2, NK, 256], BF16, name=f"cs{s}", tag=f"cs{s}")
            for s in range(NS)]

    def load_seq(t, d, s, pieces=1):
        for p in range(pieces):
            kw = NK // pieces
            nc.sync.dma_start(
                out=t[s][:, p * kw:(p + 1) * kw, :],
                in_=d.ap()[s * 128:(s + 1) * 128, p * kw * 512:(p + 1) * kw * 512],
            )

    def load_cs(s, halves=(0, 1)):
        for hf in halves:
            nc.sync.dma_start(
                out=cs_t[s][:, hf, :, :],
                in_=cs_d.ap()[s * 128:(s + 1) * 128, hf * 2048:(hf + 1) * 2048],
            )

    wq_s = wpool.tile([128, NK, 128], BF16, name="wq_s")
    wk_s = wpool.tile([128, NK, 128], BF16, name="wk_s")
    wv_s = wpool.tile([128, NK, 128], BF16, name="wv_s")
    wo_s = wpool.tile([128, D], BF16, name="wo_s")
    # all on the sync HWDGE ring: descriptors are served in dispatch order,
    # so this order IS the arrival priority.
    nc.sync.dma_start(out=wk_s, in_=wk_d.ap())
    nc.sync.dma_start(out=wv_s, in_=wv_d.ap())
    load_cs(0, halves=(0,))
    nc.sync.dma_start(out=wq_s, in_=wq_d.ap())
    load_seq(xs_t, xs_d, 0, pieces=2)
    load_cs(0, halves=(1,))
    load_cs(1)
    load_cs(2)
    load_cs(3)
    load_seq(xs_t, xs_d, 1)
    load_seq(xs_t, xs_d, 2)
    load_seq(xs_t, xs_d, 3)
    nc.sync.dma_start(out=wo_s, in_=wo_d.ap())
    # ---- PE HAM warm-up (junk matmuls, no data deps) ----
    # lives in the ps_st ring so it never blocks the kv/q accumulators
    warm_ps = ps_st.tile([128, 1024], F32, name="warm_ps", tag="st")
    for _ in range(NWARM):
        nc.tensor.matmul(warm_ps[:, 0:512], junkw, junkm, start=True, stop=True)

    # ---- big SBUF tensors ----
    qT = big.tile([128, SEQ], BF16, name="qT", tag="qT")
    kT = big.tile([128, SEQ], BF16, name="kT", tag="kT")
    vT = big.tile([128, SEQ], F32, name="vT", tag="vT")
    OT = big.tile([128, SEQ], BF16, name="OT", tag="OT")
    # V per head+m-chunk, with a ones column (65th) that accumulates the
    # softmax denominators during the PV matmul.
    Vall = big.tile([128, 2, NM, VPAD], BF16, name="Vall", tag="Vall")
    nc.vector.memset(Vall, 0.0)
    ones_sb = wpool.tile([128, 2 * NM], F32, name="ones_sb")
    nc.vector.memset(ones_sb, 1.0)
    nc.vector.tensor_copy(
        out=Vall[:, :, :, DH:DH + 1],
        in_=ones_sb.rearrange("p (h m o) -> p h m o", h=2, o=1),
    )

    # ---- compute emitters ----
    def q_proj_mms(s, ks):
        """Partial q projection: chunks ks of the contraction accumulate."""
        nonlocal q_acc
        if ks[0] == 0:
            q_acc = ps_small.tile([128, 512], F32, name="q_acc", tag="small")
        for k in ks:
            nc.tensor.matmul(
                q_acc, wq_s[:, k, :], xs_t[s][:, k, :],
                start=(k == 0), stop=(k == NK - 1),
            )
        if ks[-1] == NK - 1:
            nc.vector.tensor_copy(out=qT[:, s * 512:(s + 1) * 512], in_=q_acc)

    def kv_proj_mms(g, ks):
        nonlocal k_acc, v_acc
        if ks[0] == 0:
            k_acc = ps_small.tile([128, 512], F32, name="k_acc", tag="small")
            v_acc = ps_small.tile([128, 512], F32, name="v_acc", tag="small")
        for k in ks:
            # moving spans both 256-col halves via a strided AP, so each
            # accumulator keeps a single PSUM group per bank
            nc.tensor.matmul(
                k_acc, wk_s[:, k, :], cs_t[g][:, :, k, :],
                start=(k == 0), stop=(k == NK - 1),
            )
            nc.tensor.matmul(
                v_acc, wv_s[:, k, :], cs_t[g][:, :, k, :],
                start=(k == 0), stop=(k == NK - 1),
            )
        if ks[-1] == NK - 1:
            nc.vector.tensor_copy(out=kT[:, g * 512:(g + 1) * 512], in_=k_acc)
            nc.vector.tensor_copy(out=vT[:, g * 512:(g + 1) * 512], in_=v_acc)

    def kv_proj_half(g, hf):
        """One 256-col half of a kv group (half-width accumulators)."""
        ka_h = ps_small.tile([128, 256], F32, name="ka_h", tag="small")
        va_h = ps_small.tile([128, 256], F32, name="va_h", tag="small")
        for k in range(NK):
            nc.tensor.matmul(
                ka_h, wk_s[:, k, :], cs_t[g][:, hf, k, :],
                start=(k == 0), stop=(k == NK - 1),
            )
            nc.tensor.matmul(
                va_h, wv_s[:, k, :], cs_t[g][:, hf, k, :],
                start=(k == 0), stop=(k == NK - 1),
            )
        n0 = g * 512 + hf * 256
        nc.vector.tensor_copy(out=kT[:, n0:n0 + 256], in_=ka_h)
        nc.vector.tensor_copy(out=vT[:, n0:n0 + 256], in_=va_h)

    q_acc = k_acc = v_acc = None

    def v_transpose(g, half=None):
        """Vall[:, h, mc, 0:64] = vT[h*64:(h+1)*64, mc*128:(mc+1)*128].T

        Each transpose round-trips PE->PSUM->DVE through a 2-buffer pool
        (~0.9us each), so a full group is a ~3.5us latency chain; `half`
        splits it across two fill slots."""
        mcs = range(4 * g, 4 * g + 4) if half is None else \
            range(4 * g + 2 * half, 4 * g + 2 * half + 2)
        for mc in mcs:
            # one [128,128] transpose covers BOTH heads: out cols 0:64 are
            # head0 features, 64:128 head1 -> one contiguous-split copy
            tp = ps_small.tile([128, 128], F32, name="tp", tag="small")
            nc.tensor.transpose(
                tp, vT[:, mc * 128:(mc + 1) * 128], ident,
            )
            nc.vector.tensor_copy(
                out=Vall[:, :, mc, 0:DH],
                in_=tp.rearrange("p (h d) -> p h d", h=2),
            )

    def st_mm(s, mc):
        n0, n1 = s * 512, (s + 1) * 512
        m0, m1 = mc * 128, (mc + 1) * 128
        st = ps_st.tile([128, 1024], F32, name="st", tag="st")
        nc.tensor.matmul(
            st[:, 0:512], kT[0:DH, m0:m1], qT[0:DH, n0:n1],
            start=True, stop=True, tile_position=(0, 0),
        )
        nc.tensor.matmul(
            st[:, 512:1024], kT[DH:128, m0:m1], qT[DH:128, n0:n1],
            start=True, stop=True, tile_position=(64, 0),
        )
        return st

    def act_exp(st):
        pt = ptp.tile([128, 1024], BF16, name="pt", tag="pt")
        nc.scalar.activation(
            out=pt, in_=st,
            func=mybir.ActivationFunctionType.Exp,
            bias=zbias, scale=SCALE,
        )
        return pt

    def pv_mm(oaug, mc, pt):
        nc.tensor.matmul(
            oaug[0], Vall[:, 0, mc, 0:DH + 1], pt[:, 0:512],
            start=(mc == 0), stop=(mc == NM - 1),
        )
        nc.tensor.matmul(
            oaug[1], Vall[:, 1, mc, 0:DH + 1], pt[:, 512:1024],
            start=(mc == 0), stop=(mc == NM - 1),
        )

    def mk_oaug(s):
        return [
            ps_oaug.tile([DH + 1, 512], F32, name=f"oaug{s}_{h}", tag="oaug")
            for h in range(2)
        ]

    def attn_s(s, fills, fill_first=()):
        """One n-chunk of attention; fills[i] emits PE filler work that is
        injected into slot i so it runs while ScalarE chews on exp.
        Slots in fill_first emit the fill BEFORE S^T(i+1) — required when
        the fill writes the kT columns that S^T reads (tile deps are
        built in emission order); everywhere else S^T goes first so the
        exp stream is never queued behind filler matmuls."""
        oaug = mk_oaug(s)
        sts = [None, None]
        pts = [None, None]
        sts[0] = st_mm(s, 0)
        pts[0] = act_exp(sts[0])
        for mc in range(NM):
            fill = fills[mc] if mc < len(fills) else None
            if fill is not None and mc in fill_first:
                fill()
            if mc < NM - 1:
                sts[(mc + 1) % 2] = st_mm(s, mc + 1)
                pts[(mc + 1) % 2] = act_exp(sts[(mc + 1) % 2])
            if fill is not None and mc not in fill_first:
                fill()
            pv_mm(oaug, mc, pts[mc % 2])
        # eager PSUM evacuation: frees the oaug banks for the next n-chunk
        oaug_sb = []
        for h in range(2):
            t = ostage.tile([DH + 1, 512], F32, name="oaug_sb", tag="oaug_sb")
            nc.vector.tensor_copy(out=t, in_=oaug[h])
            oaug_sb.append(t)
        # both heads' [1,512] denominator rows scattered into ONE [128,8]
        # tile (SBUF->SBUF DMA) so one reciprocal + one bounce serves both.
        # A wide-free-dim reciprocal measures ~3.3us and would block the
        # in-order DVE stream, hence the [128,x] layout.
        den_p = ostage.tile([128, 2, 4], F32, name="den_p", tag="den_p")
        for h in range(2):
            nc.sync.dma_start(
                out=den_p[:, h, :], in_=oaug_sb[h][DH:DH + 1, :]
            )
        return den_p, oaug_sb

    def fin_rest(s, den_p, oaug_sb):
        """Normalize by softmax denominators into OT: reciprocal on the
        [128,8] layout, then one DRAM-bounce broadcast to [64, 1024]."""
        n0, n1 = s * 512, (s + 1) * 512
        rec_p = ostage.tile([128, 2, 4], F32, name="rec_p", tag="rec_p")
        nc.vector.reciprocal(out=rec_p, in_=den_p)
        rec_b = ostage.tile([128, 2, 4], BF16, name="rec_b", tag="rec_b")
        nc.vector.tensor_copy(out=rec_b, in_=rec_p)
        scr = dscr.tile([128, 8], BF16, name="scr", tag="scr")
        nc.sync.dma_start(out=scr, in_=rec_b)
        rep = ostage.tile([DH, 1024], BF16, name="rep", tag="rep")
        nc.sync.dma_start(
            out=rep,
            in_=scr.rearrange("p f -> (p f)").partition_broadcast(DH),
        )
        # rep linear layout per column: p*8 + h*4 + j with n = 4p+j
        rep_h = rep.rearrange("q (p h j) -> q h p j", h=2, j=4)
        for h in range(2):
            nc.vector.tensor_mul(
                out=OT[h * DH:(h + 1) * DH, n0:n1].rearrange(
                    "q (p j) -> q p j", j=4),
                in0=oaug_sb[h][0:DH, :].rearrange("q (p j) -> q p j", j=4),
                in1=rep_h[:, h, :, :],
            )

    def outproj_tile(s, t, n_dma_pieces=1, tail=False):
        nt = s * 4 + t
        osb = opool.tile([128, 1024], BF16, name="osb", tag="osb")
        for half in range(2):
            c0, c1 = half * 512, (half + 1) * 512
            ops = ps_small.tile([128, 512], F32, name="ops", tag="small")
            nc.tensor.matmul(
                ops, OT[:, nt * 128:(nt + 1) * 128], wo_s[:, c0:c1],
                start=True, stop=True,
            )
            if tail and half == 0:
                nc.scalar.copy(out=osb[:, c0:c1], in_=ops)
            else:
                nc.vector.tensor_copy(out=osb[:, c0:c1], in_=ops)
        for p in range(n_dma_pieces):
            w = 1024 // n_dma_pieces
            nc.sync.dma_start(
                out=out_d.ap()[nt * 128:(nt + 1) * 128, p * w:(p + 1) * w],
                in_=osb[:, p * w:(p + 1) * w],
            )

    # ---- schedule ----
    # only half 0 of kv(0) before the exp stream starts; half 1 is slot-0
    # fill work (S^T(0,0..1) only needs kT cols 0:256)
    kv_proj_half(0, 0)
    q_proj_mms(0, list(range(NK)))

    # s=0 fills: v-transposes (split in halves — PSUM latency chains),
    # chasing kv projections of groups 1..3, then q(1).  PV(0,mc) only
    # needs Vall up to mc, so half b of group g lands one slot later.
    fills0 = [None] * NM
    fills0[0] = lambda: (kv_proj_half(0, 1), v_transpose(0, 0))
    fills0[1] = lambda: (v_transpose(0, 1), kv_proj_mms(1, [0, 1]))
    fills0[2] = lambda: kv_proj_mms(1, [2, 3, 4])
    fills0[3] = lambda: kv_proj_mms(1, [5, 6, 7])
    fills0[4] = lambda: v_transpose(1, 0)
    fills0[5] = lambda: (v_transpose(1, 1), kv_proj_mms(2, [0, 1]))
    fills0[6] = lambda: kv_proj_mms(2, [2, 3, 4])
    fills0[7] = lambda: kv_proj_mms(2, [5, 6, 7])
    fills0[8] = lambda: v_transpose(2, 0)
    fills0[9] = lambda: (v_transpose(2, 1), kv_proj_mms(3, [0, 1]))
    fills0[10] = lambda: kv_proj_mms(3, [2, 3, 4])
    fills0[11] = lambda: kv_proj_mms(3, [5, 6, 7])
    fills0[12] = lambda: v_transpose(3, 0)
    fills0[13] = lambda: (v_transpose(3, 1), q_proj_mms(1, [0, 1, 2]))
    fills0[14] = lambda: q_proj_mms(1, [3, 4, 5])
    fills0[15] = lambda: q_proj_mms(1, [6, 7])

    def mk_fills(qs, op_s):
        """Fills for attn chunk s>=1: q projection of chunk qs early,
        out-projection of chunk op_s late (after its fin completes)."""
        f = [None] * NM
        if qs is not None:
            f[1] = lambda: q_proj_mms(qs, [0, 1, 2, 3])
            f[2] = lambda: q_proj_mms(qs, [4, 5, 6, 7])
        if op_s is not None:
            for i, t in enumerate(range(4)):
                f[10 + i] = (lambda tt: lambda: outproj_tile(op_s, tt))(t)
        return f

    den_p, oaug_sb = attn_s(0, fills0, fill_first={3, 7, 11})
    fin_rest(0, den_p, oaug_sb)
    den_p, oaug_sb = attn_s(1, mk_fills(2, 0))
    fin_rest(1, den_p, oaug_sb)
    den_p, oaug_sb = attn_s(2, mk_fills(3, 1))
    fin_rest(2, den_p, oaug_sb)
    den_p, oaug_sb = attn_s(3, mk_fills(None, 2))
    # keepalive: PE stays busy through the fin(3) DMA chain so the HAM
    # clock gate doesn't drop to 1.2 GHz before the final out-projection
    ka = ps_st.tile([128, 1024], F32, name="ka", tag="st")
    for _ in range(16):
        nc.tensor.matmul(ka[:, 0:512], junkw, junkm, start=True, stop=True)
    fin_rest(3, den_p, oaug_sb)
    ka2 = ps_st.tile([128, 1024], F32, name="ka2", tag="st")
    for _ in range(16):
        nc.tensor.matmul(ka2[:, 0:512], junkw, junkm, start=True, stop=True)
    for t in range(4):
        outproj_tile(3, t, n_dma_pieces=2, tail=True)

    ctx.close()


_NC = None


def _get_nc():
    global _NC
    if _NC is None:
        _NC = build_nc()
    return _NC


def _bf16():
    import ml_dtypes

    return ml_dtypes.bfloat16


def _swizzle_w(w):
    """[1024, 128] -> [128, 8*128]: chunk k of the contraction dim lands in
    column block k, so the device DMA is fully contiguous."""
    return np.ascontiguousarray(
        np.asarray(w, np.float32).reshape(NK, 128, F).transpose(1, 0, 2)
        .reshape(128, NK * F).astype(_bf16())
    )


def _swizzle_act(aT):
    """[1024, 2048] -> [512, 4096] bf16 with row s*128+p, col k*512+j =
    aT[k*128+p, s*512+j]: seq-chunk s is rows [s*128, (s+1)*128) with
    fully contiguous 8KB lines."""
    return np.ascontiguousarray(
        aT.reshape(NK, 128, NS, 512).transpose(2, 1, 0, 3)
        .reshape(NS * 128, NK * 512).astype(_bf16())
    )


def _swizzle_act_c(aT):
    """Like _swizzle_act but half-chunk-major: col half*2048 + k*256 + j =
    aT[k*128+p, s*512 + half*256 + j], so each 256-col half of a seq
    chunk is one contiguous-line DMA (lets kv(0) start on half 0)."""
    return np.ascontiguousarray(
        aT.reshape(NK, 128, NS, 2, 256).transpose(2, 1, 3, 0, 4)
        .reshape(NS * 128, NK * 512).astype(_bf16())
    )


def shard_inputs(x, context, Wq, Wk, Wv, Wo, bo):
    x = np.asarray(x, np.float32)
    context = np.asarray(context, np.float32)
    Wq = np.asarray(Wq, np.float32)
    Wk = np.asarray(Wk, np.float32)
    Wv = np.asarray(Wv, np.float32)
    Wo = np.asarray(Wo, np.float32)

    xs = [_swizzle_act(np.ascontiguousarray(x[b].T)) for b in range(x.shape[0])]
    cs = [_swizzle_act_c(np.ascontiguousarray(context[b].T))
          for b in range(context.shape[0])]
    in_maps = []
    for c in range(8):
        b, hp = divmod(c, 4)
        f0 = hp * F
        in_maps.append(
            {
                "xs": xs[b],
                "cs": cs[b],
                "wq": _swizzle_w(Wq[:, f0:f0 + F]),
                "wk": _swizzle_w(Wk[:, f0:f0 + F]),
                "wv": _swizzle_w(Wv[:, f0:f0 + F]),
                "wo": np.ascontiguousarray(Wo[f0:f0 + F, :]).astype(_bf16()),
            }
        )
    return in_maps


def kernel(x, context, Wq, Wk, Wv, Wo, bo):
    from concourse.bass_utils import run_bass_kernel_spmd

    in_maps = shard_inputs(x, context, Wq, Wk, Wv, Wo, bo)
    nc = _get_nc()
    res = run_bass_kernel_spmd(nc, in_maps, list(range(8)))
    out = np.zeros((2, SEQ, D), np.float32)
    for c in range(8):
        out[c // 4] += np.asarray(res.results[c]["out_p"], np.float32)
    out += np.asarray(bo, np.float32).reshape(1, 1, D)
    return out
